# revision 1
# baseline (speedup 1.0000x reference)
"""Trainium2 kernel for nn_Attention_28492813041691.

Sharding: data-parallel over batch across the 8 NeuronCores (one sample
per core); all parameters replicated. The per-sample computation
(1x1 qkv conv -> 3x {dilated depthwise 3x3 conv -> channel-wise
multi-head attention} -> 1x1 proj conv) is compiled with XLA-Neuron and
executed SPMD via jax.pmap on the 8 axon-tunneled trn2 cores.

Self-contained: hardcodes shapes from the problem spec
(x [8,192,128,128] f32, w_qkv [576,192], w_dw* [576,1,3,3],
w_proj [192,576], temperature [8,1,1]).
"""

import numpy as np

DILATIONS = (1, 2, 3)
NUM_HEADS = 8
EPS = 1e-12

_compiled = {}


def _build():
    import jax
    import jax.numpy as jnp

    def _dwconv(x, w, d):
        # x: [C,h,w] single sample; use NCHW with batch dim of 1
        x4 = x[None]
        y = jax.lax.conv_general_dilated(
            x4, w, window_strides=(1, 1), padding=[(d, d), (d, d)],
            rhs_dilation=(d, d), dimension_numbers=('NCHW', 'OIHW', 'NCHW'),
            feature_group_count=x.shape[0])
        return y[0]

    def _attn_branch(qkv, temperature):
        # qkv: [3*dim, h, w] single sample
        c3, h, w = qkv.shape
        q, k, v = jnp.split(qkv, 3, axis=0)
        resh = lambda t: t.reshape(NUM_HEADS, -1, h * w)
        q, k, v = resh(q), resh(k), resh(v)
        q = q / jnp.maximum(jnp.linalg.norm(q, axis=-1, keepdims=True), EPS)
        k = k / jnp.maximum(jnp.linalg.norm(k, axis=-1, keepdims=True), EPS)
        attn = jnp.einsum('hcn,hdn->hcd', q, k) * temperature
        attn = jax.nn.softmax(attn, axis=-1)
        out = jnp.einsum('hcd,hdn->hcn', attn, v)
        return out.reshape(-1, h, w)

    def per_sample(x, w_qkv, w_dw1, w_dw2, w_dw3, w_proj, temperature):
        # x: [dim, h, w] one sample on one core
        qkv = jnp.einsum('chw,oc->ohw', x, w_qkv)
        dws = (w_dw1, w_dw2, w_dw3)
        outs = [_attn_branch(_dwconv(qkv, dws[i], DILATIONS[i]), temperature)
                for i in range(3)]
        out_concat = jnp.concatenate(outs, axis=0)
        return jnp.einsum('chw,oc->ohw', out_concat, w_proj)

    fn = jax.pmap(per_sample, axis_name='b',
                  in_axes=(0, None, None, None, None, None, None))
    return fn


def kernel(x, w_qkv, w_dw1, w_dw2, w_dw3, w_proj, temperature):
    if 'fn' not in _compiled:
        _compiled['fn'] = _build()
    fn = _compiled['fn']
    out = fn(np.asarray(x), np.asarray(w_qkv), np.asarray(w_dw1),
             np.asarray(w_dw2), np.asarray(w_dw3), np.asarray(w_proj),
             np.asarray(temperature))
    return np.asarray(out).astype(np.float32)


# revision 2
# speedup vs baseline: 35.7403x; 35.7403x over previous
"""Trainium2 kernel for nn_Attention_28492813041691.

Sharding: data-parallel over batch across the 8 NeuronCores (one sample
per core); all parameters (conv weights, temperature) replicated — the
attention and convs are independent per sample, so no collectives are
needed. The per-sample computation (1x1 qkv conv -> 3x {dilated
depthwise 3x3 conv -> channel-wise multi-head attention} -> 1x1 proj
conv) is compiled for the NeuronCores and executed SPMD over the 8
axon-tunneled trn2 cores via jax.pmap.

Kernel-level choices vs the naive graph:
 - pointwise (1x1) convs expressed as plain [O,C]x[C,H*W] matmuls
   (PE-friendly, avoids conv lowering),
 - depthwise dilated 3x3 conv expressed as 9 shifted multiply-adds on
   the padded tensor (avoids XLA grouped-conv lowering with 576 groups,
   maps to DVE/ACT elementwise units),
 - channel-attention (per-head 24x24 Gram + softmax) left in fp32.

Self-contained: hardcodes shapes from the problem spec
(x [8,192,128,128] f32, w_qkv [576,192], w_dw* [576,1,3,3],
w_proj [192,576], temperature [8,1,1]).
"""

import numpy as np

DILATIONS = (1, 2, 3)
NUM_HEADS = 8
EPS = 1e-12

_cache = {}


def _build():
    import jax
    import jax.numpy as jnp

    def dwconv_shift(q2, w, d):
        # q2: [C, h, w]; w: [C,1,3,3]; dilation d, 'same' zero padding.
        C, h, wd = q2.shape
        qp = jnp.pad(q2, ((0, 0), (d, d), (d, d)))
        acc = None
        for iy in range(3):
            for ix in range(3):
                t = qp[:, iy * d:iy * d + h, ix * d:ix * d + wd] \
                    * w[:, 0, iy, ix][:, None, None]
                acc = t if acc is None else acc + t
        return acc

    def per_sample(x, w_qkv, w_dw1, w_dw2, w_dw3, w_proj, temperature):
        C, h, w = x.shape
        qkv = (w_qkv @ x.reshape(C, h * w)).reshape(576, h, w)
        outs = []
        for dw, d in zip((w_dw1, w_dw2, w_dw3), DILATIONS):
            y = dwconv_shift(qkv, dw, d)
            q, k, v = jnp.split(y.reshape(576, h * w), 3, axis=0)
            r = lambda t: t.reshape(NUM_HEADS, -1, h * w)
            q, k, v = r(q), r(k), r(v)
            q = q / jnp.maximum(
                jnp.linalg.norm(q, axis=-1, keepdims=True), EPS)
            k = k / jnp.maximum(
                jnp.linalg.norm(k, axis=-1, keepdims=True), EPS)
            attn = jax.nn.softmax(
                jnp.einsum('hcn,hdn->hcd', q, k) * temperature, axis=-1)
            outs.append(
                jnp.einsum('hcd,hdn->hcn', attn, v).reshape(-1, h * w))
        out = w_proj @ jnp.concatenate(outs, axis=0)
        return out.reshape(192, h, w)

    return jax.pmap(per_sample)


def kernel(x, w_qkv, w_dw1, w_dw2, w_dw3, w_proj, temperature):
    import jax
    if 'fn' not in _cache:
        _cache['fn'] = _build()
        _cache['devs'] = jax.devices()[:8]
    fn, devs = _cache['fn'], _cache['devs']

    x = np.ascontiguousarray(np.asarray(x, dtype=np.float32))
    xs = jax.device_put_sharded([x[i] for i in range(8)], devs)
    reps = [jax.device_put_replicated(np.asarray(a, dtype=np.float32), devs)
            for a in (w_qkv, w_dw1, w_dw2, w_dw3, w_proj, temperature)]
    out = fn(xs, *reps)
    return np.asarray(out).astype(np.float32)
